# revision 1
# baseline (speedup 1.0000x reference)
"""BitLinear (ternary weight) inference kernel for Trainium2, 8-core SPMD.

Full-input contract: kernel(**inputs) takes the complete tensors and returns
the complete output. Internally the batch dim (B=8) is sharded 1:1 onto the
8 NeuronCores; each core computes y[b] = x[b] @ (w_q * 2^s_exp)^T + bias as
a 2048x2048x2048 bf16 matmul with fp32 accumulation.

Host-side prep (cheap, O(bytes)): fold the power-of-two per-channel scale
into the ternary weights (exact in bf16), transpose w to [IN, OUT], cast x
to bf16, broadcast bias to [128, OUT]. The x transpose needed for the PE
contraction layout is done on-device by DMA-transpose loads.
"""
import os

import ml_dtypes
import numpy as np

B, T, IN, OUT = 8, 2048, 2048, 2048
P = 128
NCORES = 8
NF = 512  # matmul free dim (one PSUM bank of fp32)

last_exec_time_ns = None
_CACHE = {}


def _install_prof_shim():
    """Make antenv.axon_hooks importable so trace=True works under axon."""
    import sys
    import types

    if "antenv.axon_hooks" in sys.modules:
        return
    try:
        from trn_agent_boot.trn_boot import _ntff_profile_via_ctypes
    except ImportError:
        return
    hook = _ntff_profile_via_ctypes("/opt/axon/libaxon_pjrt.so")
    mod = types.ModuleType("antenv.axon_hooks")
    mod.get_axon_ntff_profile_hook = lambda: hook
    mod.set_axon_ntff_profile_hook = lambda h: None
    sys.modules["antenv.axon_hooks"] = mod


def _build():
    import concourse.bacc as bacc
    import concourse.mybir as mybir
    from concourse.tile import TileContext

    nc = bacc.Bacc()
    x = nc.dram_tensor("x", (T, IN), mybir.dt.bfloat16, kind="ExternalInput")
    w = nc.dram_tensor("w", (IN, OUT), mybir.dt.bfloat16, kind="ExternalInput")
    bias = nc.dram_tensor("bias", (P, OUT), mybir.dt.float32, kind="ExternalInput")
    y = nc.dram_tensor("y", (T, OUT), mybir.dt.float32, kind="ExternalOutput")

    KT = IN // P   # contraction chunks
    TT = T // P    # output row tiles
    OC = OUT // NF  # psum banks per row tile

    with TileContext(nc) as tc:
        with tc.tile_pool(name="wp", bufs=1) as wp, \
             tc.tile_pool(name="xp", bufs=1) as xp, \
             tc.tile_pool(name="bp", bufs=1) as bp, \
             tc.tile_pool(name="op", bufs=3) as op_, \
             tc.tile_pool(name="pp", bufs=2, space="PSUM") as pp:

            bias_t = bp.tile([P, OUT], mybir.dt.float32, tag="bias")
            nc.sync.dma_start(bias_t, bias[:, :])

            # Whole wT resident in SBUF: 16 tiles [128(i), OUT] bf16 = 8 MiB.
            w_tiles = []
            for k in range(KT):
                wt = wp.tile([P, OUT], mybir.dt.bfloat16, tag=f"w{k}")
                nc.sync.dma_start(wt, w[k * P:(k + 1) * P, :])
                w_tiles.append(wt)

            # Whole x^T resident in SBUF via DMA-transpose loads:
            # 16 tiles [128(i), T] bf16 = 8 MiB.
            xT_tiles = []
            for k in range(KT):
                xt = xp.tile([P, T], mybir.dt.bfloat16, tag=f"x{k}")
                nc.sync.dma_start_transpose(xt, x[:, k * P:(k + 1) * P])
                xT_tiles.append(xt)

            for tt in range(TT):
                ps = pp.tile([P, OUT], mybir.dt.float32, tag="ps")
                for k in range(KT):
                    lhsT = xT_tiles[k][:, tt * P:(tt + 1) * P]
                    for oc in range(OC):
                        nc.tensor.matmul(
                            ps[:, oc * NF:(oc + 1) * NF],
                            lhsT,
                            w_tiles[k][:, oc * NF:(oc + 1) * NF],
                            start=(k == 0),
                            stop=(k == KT - 1),
                        )
                ot = op_.tile([P, OUT], mybir.dt.float32, tag="out")
                for oc in range(OC):
                    sl = slice(oc * NF, (oc + 1) * NF)
                    nc.vector.tensor_add(ot[:, sl], ps[:, sl], bias_t[:, sl])
                # ACT's HWDGE ring so stores don't queue behind input loads.
                nc.scalar.dma_start(y[tt * P:(tt + 1) * P, :], ot)

    nc.compile()
    return nc


def kernel(x, w_q, s_exp, bias):
    global last_exec_time_ns
    from concourse.bass_utils import run_bass_kernel_spmd

    x = np.asarray(x)
    w_q = np.asarray(w_q)
    s_exp = np.asarray(s_exp)
    bias = np.asarray(bias, dtype=np.float32)
    assert x.shape == (B, T, IN) and w_q.shape == (OUT, IN)

    # Fold the power-of-two per-output-channel scale into the ternary
    # weights: values are +-2^s or 0, exact in bf16.
    scale = np.exp2(s_exp.astype(np.float32))
    w_scaled_t = (w_q.astype(np.float32) * scale[:, None]).T
    w_bf16 = np.ascontiguousarray(w_scaled_t).astype(ml_dtypes.bfloat16)
    bias_bcast = np.ascontiguousarray(
        np.broadcast_to(bias.astype(np.float32), (P, OUT)))
    x_bf16 = x.astype(ml_dtypes.bfloat16)

    nc = _CACHE.get("nc")
    if nc is None:
        nc = _CACHE["nc"] = _build()

    in_maps = [
        {"x": x_bf16[b], "w": w_bf16, "bias": bias_bcast} for b in range(B)
    ]

    trace = bool(int(os.environ.get("BITLIN_TRACE", "0")))
    if trace:
        _install_prof_shim()
    res = run_bass_kernel_spmd(nc, in_maps, list(range(NCORES)), trace=trace)
    last_exec_time_ns = res.exec_time_ns

    out = np.stack([res.results[b]["y"] for b in range(B)], axis=0)
    return out.astype(np.float32, copy=False)


# revision 2
# speedup vs baseline: 1.0483x; 1.0483x over previous
"""BitLinear (ternary weight) inference kernel for Trainium2, 8-core SPMD.

Full-input contract: kernel(**inputs) takes the complete tensors and returns
the complete output. The batch dim (B=8) is sharded 1:1 onto the 8
NeuronCores; each core computes y[b] = x[b] @ (w_q * 2^s_exp)^T + bias as a
2048^3 bf16 matmul with fp32 accumulation.

Host prep (cheap, O(bytes)): fold the power-of-two per-channel scale into
the ternary weights (exact in bf16), pre-transpose both operands into the
PE's contraction-major layout, cast x to bf16, broadcast bias to [128, OUT].

Device schedule: the k-contraction is split into two passes (k-chunks 0..3
and 4..15). Pass A only needs 3 MiB of inputs, so the PE starts ~8us in;
its partial sums go to HBM via plain SWDGE stores, and pass B accumulates
on top with accum_op=add DMAs on the same SWDGE ring (FIFO-ordered, so the
read-modify-write is safe). Inputs stream on the Sync HWDGE ring meanwhile.
"""
import os

import ml_dtypes
import numpy as np

B, T, IN, OUT = 8, 2048, 2048, 2048
P = 128
NCORES = 8
NF = 512        # matmul free dim (one PSUM bank of fp32)
KA = 4          # k-chunks in pass A (first-pass dependency set = KA MiB won't gate PE)

last_exec_time_ns = None
_CACHE = {}


def _install_prof_shim():
    """Make antenv.axon_hooks importable so trace=True works under axon."""
    import sys
    import types

    if "antenv.axon_hooks" in sys.modules:
        return
    try:
        from trn_agent_boot.trn_boot import _ntff_profile_via_ctypes
    except ImportError:
        return
    hook = _ntff_profile_via_ctypes("/opt/axon/libaxon_pjrt.so")
    mod = types.ModuleType("antenv.axon_hooks")
    mod.get_axon_ntff_profile_hook = lambda: hook
    mod.set_axon_ntff_profile_hook = lambda h: None
    sys.modules["antenv.axon_hooks"] = mod


def _build():
    import concourse.bacc as bacc
    import concourse.mybir as mybir
    from concourse.tile import TileContext

    nc = bacc.Bacc()
    x = nc.dram_tensor("x", (IN, T), mybir.dt.bfloat16, kind="ExternalInput")
    w = nc.dram_tensor("w", (IN, OUT), mybir.dt.bfloat16, kind="ExternalInput")
    bias = nc.dram_tensor("bias", (P, OUT), mybir.dt.float32, kind="ExternalInput")
    y = nc.dram_tensor("y", (T, OUT), mybir.dt.float32, kind="ExternalOutput")

    KT = IN // P    # contraction chunks
    TT = T // P     # output row tiles
    OC = OUT // NF  # psum banks per row tile

    with TileContext(nc) as tc:
        with tc.tile_pool(name="wp", bufs=1) as wp, \
             tc.tile_pool(name="xp", bufs=1) as xp, \
             tc.tile_pool(name="bp", bufs=1) as bp, \
             tc.tile_pool(name="op", bufs=3) as op_, \
             tc.tile_pool(name="pp", bufs=2, space="PSUM") as pp:

            bias_t = bp.tile([P, OUT], mybir.dt.float32, tag="bias")
            nc.sync.dma_start(bias_t, bias[:, :])

            # Interleave w/x chunk loads k-wise so pass A's working set
            # (k < KA) lands first and the PE can start after ~3 MiB.
            w_tiles = [None] * KT
            xT_tiles = [None] * KT
            for k in range(KT):
                wt = wp.tile([P, OUT], mybir.dt.bfloat16, tag=f"w{k}")
                nc.sync.dma_start(wt, w[k * P:(k + 1) * P, :])
                w_tiles[k] = wt
                xt = xp.tile([P, T], mybir.dt.bfloat16, tag=f"x{k}")
                nc.sync.dma_start(xt, x[k * P:(k + 1) * P, :])
                xT_tiles[k] = xt

            def half_pass(k_lo, k_hi, first):
                for tt in range(TT):
                    ps = pp.tile([P, OUT], mybir.dt.float32, tag="ps")
                    for k in range(k_lo, k_hi):
                        lhsT = xT_tiles[k][:, tt * P:(tt + 1) * P]
                        for oc in range(OC):
                            nc.tensor.matmul(
                                ps[:, oc * NF:(oc + 1) * NF],
                                lhsT,
                                w_tiles[k][:, oc * NF:(oc + 1) * NF],
                                start=(k == k_lo),
                                stop=(k == k_hi - 1),
                            )
                    ot = op_.tile([P, OUT], mybir.dt.float32, tag="out")
                    for oc in range(OC):
                        sl = slice(oc * NF, (oc + 1) * NF)
                        if first:
                            nc.vector.tensor_add(ot[:, sl], ps[:, sl], bias_t[:, sl])
                        else:
                            nc.vector.tensor_copy(ot[:, sl], ps[:, sl])
                    dst = y[tt * P:(tt + 1) * P, :]
                    if first:
                        nc.gpsimd.dma_start(dst, ot)
                    else:
                        nc.gpsimd.dma_start(dst, ot, accum_op=mybir.AluOpType.add)

            half_pass(0, KA, first=True)
            half_pass(KA, KT, first=False)

    nc.compile()
    return nc


def kernel(x, w_q, s_exp, bias):
    global last_exec_time_ns
    from concourse.bass_utils import run_bass_kernel_spmd

    x = np.asarray(x)
    w_q = np.asarray(w_q)
    s_exp = np.asarray(s_exp)
    bias = np.asarray(bias, dtype=np.float32)
    assert x.shape == (B, T, IN) and w_q.shape == (OUT, IN)

    # Fold the power-of-two per-output-channel scale into the ternary
    # weights: values are +-2^s or 0, exact in bf16.
    scale = np.exp2(s_exp.astype(np.float32))
    w_scaled_t = (w_q.astype(np.float32) * scale[:, None]).T
    w_bf16 = np.ascontiguousarray(w_scaled_t).astype(ml_dtypes.bfloat16)
    bias_bcast = np.ascontiguousarray(
        np.broadcast_to(bias.astype(np.float32), (P, OUT)))
    # Contraction-major layout for the PE: x^T[b] = [IN, T], bf16.
    xT_bf16 = np.ascontiguousarray(
        x.astype(ml_dtypes.bfloat16).transpose(0, 2, 1))

    nc = _CACHE.get("nc")
    if nc is None:
        nc = _CACHE["nc"] = _build()

    in_maps = [
        {"x": xT_bf16[b], "w": w_bf16, "bias": bias_bcast} for b in range(B)
    ]

    trace = bool(int(os.environ.get("BITLIN_TRACE", "0")))
    if trace:
        _install_prof_shim()
    res = run_bass_kernel_spmd(nc, in_maps, list(range(NCORES)), trace=trace)
    last_exec_time_ns = res.exec_time_ns

    out = np.stack([res.results[b]["y"] for b in range(B)], axis=0)
    return out.astype(np.float32, copy=False)


# revision 5
# speedup vs baseline: 1.0544x; 1.0058x over previous
"""BitLinear (ternary weight) inference kernel for Trainium2, 8-core SPMD.

Full-input contract: kernel(**inputs) takes the complete tensors and returns
the complete output. The batch dim (B=8) is sharded 1:1 onto the 8
NeuronCores; each core computes y[b] = x[b] @ (w_q * 2^s_exp)^T + bias as a
2048^3 bf16 matmul with fp32 accumulation.

Host prep (cheap, O(bytes)): fold the power-of-two per-channel scale into
the ternary weights (exact in bf16), pre-transpose both operands into the
PE's contraction-major layout, cast x to bf16, broadcast bias to [128, OUT].

Device schedule: the k-contraction is split into two passes (k-chunks 0..3
and 4..15). Pass A only needs 3 MiB of inputs, so the PE starts ~8us in;
its partial sums go to HBM via plain SWDGE stores, and pass B accumulates
on top with accum_op=add DMAs on the same SWDGE ring (FIFO-ordered, so the
read-modify-write is safe). Inputs stream on the Sync HWDGE ring meanwhile.
"""
import os

import ml_dtypes
import numpy as np

B, T, IN, OUT = 8, 2048, 2048, 2048
P = 128
NCORES = 8
NF = 512        # matmul free dim (one PSUM bank of fp32)
KA = 4          # k-chunks in pass A (first-pass dependency set = KA MiB won't gate PE)

last_exec_time_ns = None
_CACHE = {}


def _install_prof_shim():
    """Make antenv.axon_hooks importable so trace=True works under axon."""
    import sys
    import types

    if "antenv.axon_hooks" in sys.modules:
        return
    try:
        from trn_agent_boot.trn_boot import _ntff_profile_via_ctypes
    except ImportError:
        return
    hook = _ntff_profile_via_ctypes("/opt/axon/libaxon_pjrt.so")
    mod = types.ModuleType("antenv.axon_hooks")
    mod.get_axon_ntff_profile_hook = lambda: hook
    mod.set_axon_ntff_profile_hook = lambda h: None
    sys.modules["antenv.axon_hooks"] = mod


def _build():
    import concourse.bacc as bacc
    import concourse.mybir as mybir
    from concourse.tile import TileContext

    nc = bacc.Bacc()
    x = nc.dram_tensor("x", (IN, T), mybir.dt.bfloat16, kind="ExternalInput")
    w = nc.dram_tensor("w", (IN, OUT), mybir.dt.bfloat16, kind="ExternalInput")
    bias = nc.dram_tensor("bias", (P, OUT), mybir.dt.float32, kind="ExternalInput")
    y = nc.dram_tensor("y", (T, OUT), mybir.dt.float32, kind="ExternalOutput")

    KT = IN // P    # contraction chunks
    TT = T // P     # output row tiles
    OC = OUT // NF  # psum banks per row tile

    with TileContext(nc) as tc:
        with tc.tile_pool(name="wp", bufs=1) as wp, \
             tc.tile_pool(name="xp", bufs=1) as xp, \
             tc.tile_pool(name="bp", bufs=1) as bp, \
             tc.tile_pool(name="op", bufs=3) as op_, \
             tc.tile_pool(name="pp", bufs=4, space="PSUM") as pp:

            # Interleave w/x chunk loads k-wise so pass A's working set
            # (k < KA) lands first and the PE can start after ~3 MiB.
            w_tiles = [None] * KT
            xT_tiles = [None] * KT
            bias_t = bp.tile([P, OUT], mybir.dt.float32, tag="bias")
            for k in range(KT):
                wt = wp.tile([P, OUT], mybir.dt.bfloat16, tag=f"w{k}")
                nc.sync.dma_start(wt, w[k * P:(k + 1) * P, :])
                w_tiles[k] = wt
                xt = xp.tile([P, T], mybir.dt.bfloat16, tag=f"x{k}")
                nc.sync.dma_start(xt, x[k * P:(k + 1) * P, :])
                xT_tiles[k] = xt
                if k == KA - 1:
                    nc.sync.dma_start(bias_t, bias[:, :])

            HOUT = OUT // 2  # two psum tiles (2 banks each) per row tile

            def half_pass(k_lo, k_hi, first):
                for tt in range(TT):
                    pss = [pp.tile([P, HOUT], mybir.dt.float32, tag="ps",
                                   name=f"ps{h}") for h in range(2)]
                    for k in range(k_lo, k_hi):
                        lhsT = xT_tiles[k][:, tt * P:(tt + 1) * P]
                        for oc in range(OC):
                            ps = pss[oc // 2]
                            lo = (oc % 2) * NF
                            nc.tensor.matmul(
                                ps[:, lo:lo + NF],
                                lhsT,
                                w_tiles[k][:, oc * NF:(oc + 1) * NF],
                                start=(k == k_lo),
                                stop=(k == k_hi - 1),
                            )
                    ot = op_.tile([P, OUT], mybir.dt.float32, tag="out")
                    for h in range(2):
                        sl = slice(h * HOUT, (h + 1) * HOUT)
                        if first:
                            nc.vector.tensor_add(ot[:, sl], pss[h], bias_t[:, sl])
                        else:
                            nc.scalar.copy(ot[:, sl], pss[h])
                    dst = y[tt * P:(tt + 1) * P, :]
                    if first:
                        nc.gpsimd.dma_start(dst, ot)
                    else:
                        nc.gpsimd.dma_start(dst, ot, accum_op=mybir.AluOpType.add)

            half_pass(0, KA, first=True)
            half_pass(KA, KT, first=False)

    nc.compile()
    return nc


def kernel(x, w_q, s_exp, bias):
    global last_exec_time_ns
    from concourse.bass_utils import run_bass_kernel_spmd

    x = np.asarray(x)
    w_q = np.asarray(w_q)
    s_exp = np.asarray(s_exp)
    bias = np.asarray(bias, dtype=np.float32)
    assert x.shape == (B, T, IN) and w_q.shape == (OUT, IN)

    # Fold the power-of-two per-output-channel scale into the ternary
    # weights: values are +-2^s or 0, exact in bf16.
    scale = np.exp2(s_exp.astype(np.float32))
    w_scaled_t = (w_q.astype(np.float32) * scale[:, None]).T
    w_bf16 = np.ascontiguousarray(w_scaled_t).astype(ml_dtypes.bfloat16)
    bias_bcast = np.ascontiguousarray(
        np.broadcast_to(bias.astype(np.float32), (P, OUT)))
    # Contraction-major layout for the PE: x^T[b] = [IN, T], bf16.
    xT_bf16 = np.ascontiguousarray(
        x.astype(ml_dtypes.bfloat16).transpose(0, 2, 1))

    nc = _CACHE.get("nc")
    if nc is None:
        nc = _CACHE["nc"] = _build()

    in_maps = [
        {"x": xT_bf16[b], "w": w_bf16, "bias": bias_bcast} for b in range(B)
    ]

    trace = bool(int(os.environ.get("BITLIN_TRACE", "0")))
    if trace:
        _install_prof_shim()
    res = run_bass_kernel_spmd(nc, in_maps, list(range(NCORES)), trace=trace)
    last_exec_time_ns = res.exec_time_ns

    out = np.stack([res.results[b]["y"] for b in range(B)], axis=0)
    return out.astype(np.float32, copy=False)


# revision 7
# speedup vs baseline: 1.0723x; 1.0169x over previous
"""BitLinear (ternary weight) inference kernel for Trainium2, 8-core SPMD.

Full-input contract: kernel(**inputs) takes the complete tensors and returns
the complete output. The batch dim (B=8) is sharded 1:1 onto the 8
NeuronCores; each core computes y[b] = x[b] @ (w_q * 2^s_exp)^T + bias as a
2048^3 bf16 matmul with fp32 accumulation.

Host prep (cheap, O(bytes)): fold the power-of-two per-channel scale into
the ternary weights (exact in bf16), pre-transpose both operands into the
PE's contraction-major layout, cast x to bf16, broadcast bias to [128, OUT].

Device schedule: the k-contraction is split into two passes (k-chunks 0..3
and 4..15). Pass A only needs 3 MiB of inputs, so the PE starts ~8us in;
its partial sums go to HBM via plain SWDGE stores, and pass B accumulates
on top with accum_op=add DMAs on the same SWDGE ring (FIFO-ordered, so the
read-modify-write is safe). Inputs stream on the Sync HWDGE ring meanwhile.
"""
import os

import ml_dtypes
import numpy as np

B, T, IN, OUT = 8, 2048, 2048, 2048
P = 128
NCORES = 8
NF = 512        # matmul free dim (one PSUM bank of fp32)
KA = 4          # k-chunks in pass A (first-pass dependency set = KA MiB won't gate PE)

last_exec_time_ns = None
_CACHE = {}


def _install_prof_shim():
    """Make antenv.axon_hooks importable so trace=True works under axon."""
    import sys
    import types

    if "antenv.axon_hooks" in sys.modules:
        return
    try:
        from trn_agent_boot.trn_boot import _ntff_profile_via_ctypes
    except ImportError:
        return
    hook = _ntff_profile_via_ctypes("/opt/axon/libaxon_pjrt.so")
    mod = types.ModuleType("antenv.axon_hooks")
    mod.get_axon_ntff_profile_hook = lambda: hook
    mod.set_axon_ntff_profile_hook = lambda h: None
    sys.modules["antenv.axon_hooks"] = mod


def _build():
    import concourse.bacc as bacc
    import concourse.mybir as mybir
    from concourse.tile import TileContext

    nc = bacc.Bacc()
    x = nc.dram_tensor("x", (IN, T), mybir.dt.bfloat16, kind="ExternalInput")
    w = nc.dram_tensor("w", (IN, OUT), mybir.dt.bfloat16, kind="ExternalInput")
    bias = nc.dram_tensor("bias", (P, OUT), mybir.dt.float32, kind="ExternalInput")
    y = nc.dram_tensor("y", (T, OUT), mybir.dt.float32, kind="ExternalOutput")

    KT = IN // P    # contraction chunks
    TT = T // P     # output row tiles
    OC = OUT // NF  # psum banks per row tile

    with TileContext(nc) as tc:
        with tc.tile_pool(name="wp", bufs=1) as wp, \
             tc.tile_pool(name="xp", bufs=1) as xp, \
             tc.tile_pool(name="bp", bufs=1) as bp, \
             tc.tile_pool(name="op", bufs=4) as op_, \
             tc.tile_pool(name="pp", bufs=4, space="PSUM") as pp:

            # Interleave w/x chunk loads k-wise so pass A's working set
            # (k < KA) lands first and the PE can start after ~3 MiB.
            w_tiles = [None] * KT
            xT_tiles = [None] * KT
            bias_t = bp.tile([P, OUT], mybir.dt.float32, tag="bias")
            for k in range(KT):
                wt = wp.tile([P, OUT], mybir.dt.bfloat16, tag=f"w{k}")
                nc.sync.dma_start(wt, w[k * P:(k + 1) * P, :])
                w_tiles[k] = wt
                xt = xp.tile([P, T], mybir.dt.bfloat16, tag=f"x{k}")
                nc.sync.dma_start(xt, x[k * P:(k + 1) * P, :])
                xT_tiles[k] = xt
                if k == KA - 1:
                    nc.sync.dma_start(bias_t, bias[:, :])

            HOUT = OUT // 2  # two psum tiles (2 banks each) per row tile
            TSPLIT = TT // 2  # row tiles 0..TSPLIT-1 two-pass, rest single-pass

            def do_tiles(tt_range, k_lo, k_hi, mode):
                # mode: "partial" = bias+store, "accum" = copy+accum-store,
                #       "single" = bias+plain store
                for tt in tt_range:
                    pss = [pp.tile([P, HOUT], mybir.dt.float32, tag="ps",
                                   name=f"ps{h}") for h in range(2)]
                    for k in range(k_lo, k_hi):
                        lhsT = xT_tiles[k][:, tt * P:(tt + 1) * P]
                        for oc in range(OC):
                            ps = pss[oc // 2]
                            lo = (oc % 2) * NF
                            nc.tensor.matmul(
                                ps[:, lo:lo + NF],
                                lhsT,
                                w_tiles[k][:, oc * NF:(oc + 1) * NF],
                                start=(k == k_lo),
                                stop=(k == k_hi - 1),
                            )
                    ot = op_.tile([P, OUT], mybir.dt.float32, tag="out")
                    for h in range(2):
                        sl = slice(h * HOUT, (h + 1) * HOUT)
                        if mode == "accum":
                            nc.scalar.copy(ot[:, sl], pss[h])
                        else:
                            nc.vector.tensor_add(ot[:, sl], pss[h], bias_t[:, sl])
                    dst = y[tt * P:(tt + 1) * P, :]
                    if mode == "partial":
                        nc.gpsimd.dma_start(dst, ot)
                    elif mode == "accum":
                        nc.gpsimd.dma_start(dst, ot, accum_op=mybir.AluOpType.add)
                    else:
                        nc.scalar.dma_start(dst, ot)

            do_tiles(range(TSPLIT), 0, KA, "partial")
            do_tiles(range(TSPLIT), KA, KT, "accum")
            do_tiles(range(TSPLIT, TT), 0, KT, "single")

    nc.compile()
    return nc


def kernel(x, w_q, s_exp, bias):
    global last_exec_time_ns
    from concourse.bass_utils import run_bass_kernel_spmd

    x = np.asarray(x)
    w_q = np.asarray(w_q)
    s_exp = np.asarray(s_exp)
    bias = np.asarray(bias, dtype=np.float32)
    assert x.shape == (B, T, IN) and w_q.shape == (OUT, IN)

    # Fold the power-of-two per-output-channel scale into the ternary
    # weights: values are +-2^s or 0, exact in bf16.
    scale = np.exp2(s_exp.astype(np.float32))
    w_scaled_t = (w_q.astype(np.float32) * scale[:, None]).T
    w_bf16 = np.ascontiguousarray(w_scaled_t).astype(ml_dtypes.bfloat16)
    bias_bcast = np.ascontiguousarray(
        np.broadcast_to(bias.astype(np.float32), (P, OUT)))
    # Contraction-major layout for the PE: x^T[b] = [IN, T], bf16.
    xT_bf16 = np.ascontiguousarray(
        x.astype(ml_dtypes.bfloat16).transpose(0, 2, 1))

    nc = _CACHE.get("nc")
    if nc is None:
        nc = _CACHE["nc"] = _build()

    in_maps = [
        {"x": xT_bf16[b], "w": w_bf16, "bias": bias_bcast} for b in range(B)
    ]

    trace = bool(int(os.environ.get("BITLIN_TRACE", "0")))
    if trace:
        _install_prof_shim()
    res = run_bass_kernel_spmd(nc, in_maps, list(range(NCORES)), trace=trace)
    last_exec_time_ns = res.exec_time_ns

    out = np.stack([res.results[b]["y"] for b in range(B)], axis=0)
    return out.astype(np.float32, copy=False)
